# revision 36
# baseline (speedup 1.0000x reference)
"""Trainium2 Bass kernel for nn_CapsuleLayer (dynamic routing).

Reference computation (B=128, I=1152, P=8, J=10, D=16):
    inputs_hat[b,i,j,d] = sum_p W[i,j,d,p] * inputs[b,i,p]
    b_logits = 0
    3x routing:
        c = softmax_j(b_logits)
        s[b,j,d] = sum_i c[b,i,j] * inputs_hat[b,i,j,d]
        outputs = squash(s)
        b_logits += sum_d inputs_hat[b,i,j,d] * outputs[b,j,d]   (iters 0,1)

Distribution: i-sharded across 8 cores (IL=144 i's per core), full batch
B=128 in the 128 SBUF partitions on every core.  Cross-core traffic is an
80KB AllReduce of the s-partials for routing iterations 0 and 1; the final
iteration's partial sum is returned per-core and reduced + squashed on the
host during the gather/unshard step (saves the third collective).

Host-side prep (part of sharding): x and W are pre-transposed and cast to
bf16 on the host into the exact SBUF layouts the PE needs —
  xt  [k=(i16 p)=128, g, b]      stationary operand k-tiles
  w2  [k, g, (d j)=160]          dense W for the iteration-0 s matmul
  wbd [k, g, 4*JD=640]           block-diagonal W expansion (4 i's per
                                 K=32 slice) for the inputs_hat matmuls
so the device does no staging at all: 3 DMA loads, then straight to
matmuls.  bf16 operands run the PE at 1 cycle/row (fp32 is 4).

Engine budget: PE does s0 + inputs_hat; DVE runs ALL routing elementwise
work (bf16 at 2x; real-HW gpsimd elementwise is far slower than the cost
model claims, so no Pool offload); ACT handles exp/sqrt, half the PSUM
drains, and scaling; the collectives are issued from gpsimd (only engine
that can) — its queue blocking during the collective aligns with the data
dependency anyway; SP triggers the bounce DMAs.
"""

import os
import sys
import functools

import numpy as np

if "/opt/trn_rl_repo" not in sys.path:
    sys.path.insert(0, "/opt/trn_rl_repo")

B = 128
I_FULL = 1152
P_DIM = 8
J = 10
D = 16
JD = D * J  # 160, flattened (d, j): col = d*J + j
NCORES = 8
ROUTINGS = 3
EPS = 1e-7

# elementwise split: DVE takes i-rows [0, SPL), Pool takes [SPL, IL).
# SPL=IL disables the Pool offload: on real HW gpsimd tensor_tensor is
# far slower than the cost model's 0.83 ns/elem (measured: offloading a
# third of the muls to Pool ADDED ~68us), so everything stays on DVE.
SPL = int(os.environ.get("K_SPL", "144"))
LBF = os.environ.get("K_LBF", "0") == "1"   # bf16 routing logits
EBF = os.environ.get("K_EBF", "0") == "1"   # bf16 softmax numerator
AGX = os.environ.get("K_AG", "0") == "1"    # AllGather + local sum


def build(n_cores, IL, repeat=1):
    """Trace + compile the SPMD Bass program (one program, all cores)."""
    import concourse.bacc as bacc
    import concourse.bass as bass
    import concourse.mybir as mybir
    import concourse.tile as tile
    from concourse.masks import make_identity

    F32 = mybir.dt.float32
    BF16 = mybir.dt.bfloat16

    assert IL % 16 == 0
    G = IL // 16

    nc = bacc.Bacc(
        "TRN2", target_bir_lowering=False, debug=False, num_devices=n_cores
    )
    xt_d = nc.dram_tensor("xt", [128, G, 128], BF16, kind="ExternalInput").ap()
    w2_d = nc.dram_tensor("w2", [128, G, JD], BF16, kind="ExternalInput").ap()
    wbd_d = nc.dram_tensor(
        "wbd", [128, G, 4 * JD], BF16, kind="ExternalInput").ap()
    out_d = nc.dram_tensor("out", [B, JD], F32, kind="ExternalOutput").ap()

    import contextlib as _ctxlib
    lp = (nc.allow_low_precision(reason="bf16 routing-state experiment")
          if (LBF or EBF) else _ctxlib.nullcontext())
    with tile.TileContext(nc, num_cores=n_cores) as tc, lp:
        for rep in range(repeat):
            _trace(tc, nc, xt_d, w2_d, wbd_d, out_d, n_cores, IL, G,
                   mybir, make_identity, rep)

    nc.compile()
    return nc


def _trace(tc, nc, xt_d, w2_d, wbd_d, out_d, n_cores, IL, G, mybir,
           make_identity, rep=0):
    import contextlib

    F32 = mybir.dt.float32
    BF16 = mybir.dt.bfloat16
    AF = mybir.ActivationFunctionType
    OP = mybir.AluOpType
    AX = mybir.AxisListType

    ctx = contextlib.ExitStack()
    with ctx:
        singles = ctx.enter_context(
            tc.tile_pool(name=f"singles{rep}", bufs=1))
        big = ctx.enter_context(tc.tile_pool(name=f"big{rep}", bufs=1))
        small = ctx.enter_context(tc.tile_pool(name=f"small{rep}", bufs=3))
        psT = ctx.enter_context(
            tc.tile_pool(name=f"psT{rep}", bufs=2, space="PSUM"))
        psS = ctx.enter_context(
            tc.tile_pool(name=f"psS{rep}", bufs=1, space="PSUM"))
        psIH = ctx.enter_context(
            tc.tile_pool(name=f"psIH{rep}", bufs=4, space="PSUM"))
        dram = ctx.enter_context(
            tc.tile_pool(name=f"dram{rep}", bufs=1, space="DRAM"))

        # ---- constants -------------------------------------------------
        ident = singles.tile([128, 128], F32)
        make_identity(nc, ident[:])
        dummy = singles.tile([128, 1], F32)
        nc.vector.memset(dummy[:], 0.0)
        eps_t = singles.tile([128, 1], F32)
        nc.vector.memset(eps_t[:], EPS)
        # Preload the natural_log_exp table set (covers Ln AND Exp): with
        # squash on Ln/Exp and softmax on Exp, no activation in the whole
        # program needs another set, so the insert_act_table_loads pass
        # should place zero further (1.28us) loads.
        if os.environ.get("K_MANLOAD", "1") == "1":
            from concourse.hw_specs import get_activation_tables
            tabs = get_activation_tables(nc.m.arch)
            set_id = list(tabs).index("natural_log_exp_and_others")
            nc.scalar.add_instruction(mybir.InstLoadActFuncSet(
                name=f"I-{nc.next_id()}", ins=[], outs=[],
                act_func_set_id=set_id))
        nc.scalar.activation(dummy[:], dummy[:], AF.Exp)

        # ---- load pre-transposed operands (3 g-groups, 3 queues) -------
        XT = big.tile([128, G, 128], BF16)
        W2 = big.tile([128, G, JD], BF16)
        WBD = big.tile([128, G, 4 * JD], BF16)
        gparts = [(i, min(i + 3, G)) for i in range(0, G, 3)]
        for g0, g1 in gparts:
            nc.sync.dma_start(out=XT[:, g0:g1, :], in_=xt_d[:, g0:g1, :])
            nc.scalar.dma_start(out=W2[:, g0:g1, :], in_=w2_d[:, g0:g1, :])
            nc.gpsimd.dma_start(out=WBD[:, g0:g1, :], in_=wbd_d[:, g0:g1, :])

        # warm the PE p-state ramp (0.65 -> 2.4 GHz needs ~3us of
        # continuous execution) with dependency-free identity transposes
        # while the input DMAs land
        for _ in range(6):
            ps_w = psT.tile([128, 128], F32, tag="pst")
            nc.tensor.transpose(ps_w[:], ident[:], ident[:])

        # ---- iteration-0 s directly from PE (c == 1/J), AllReduce ------
        # s0T[(d j), b] = sum_{(i,p)} W2[k, dj] * XT[k, b]
        ps_a = psS.tile([128, 128], F32, tag="s0a")
        ps_b = psS.tile([32, 128], F32, tag="s0b")
        for g in range(G):
            nc.tensor.matmul(ps_a[:], W2[:, g, 0:128], XT[:, g, :],
                             start=(g == 0), stop=(g == G - 1))
        for g in range(G):
            nc.tensor.matmul(ps_b[:], W2[:, g, 128:JD], XT[:, g, :],
                             start=(g == 0), stop=(g == G - 1))
        s0T_a = small.tile([128, 128], F32, tag="s0Ta")
        s0T_b = small.tile([32, 128], F32, tag="s0Tb")
        nc.scalar.mul(s0T_a[:], ps_a[:], 1.0 / J)
        nc.scalar.mul(s0T_b[:], ps_b[:], 1.0 / J)
        s0p = small.tile([128, JD], F32, tag="spart")
        pst = psT.tile([128, 128], F32, tag="pst")
        nc.tensor.transpose(pst[:], s0T_a[:], ident[:])
        nc.vector.tensor_copy(s0p[:, 0:128], pst[:])
        pstb = psT.tile([128, 32], F32, tag="pst")
        nc.tensor.transpose(pstb[:], s0T_b[:], ident[0:32, 0:32])
        nc.vector.tensor_copy(s0p[:, 128:JD], pstb[:])

        def all_reduce(s_part, tag):
            use_cc = (n_cores > 1
                      and os.environ.get("K_NO_CC", "0") != "1")
            s_glob = small.tile([128, JD], F32, tag="sglob")
            cc_in = dram.tile([B, JD], F32, name=f"ccin_{tag}_{rep}")
            nc.sync.dma_start(out=cc_in[:], in_=s_part[:])
            if use_cc and AGX:
                # AllGather + local 8-slot tree sum: real-HW AllReduce is
                # two-phase; a gather plus ~0.7us of DVE adds may beat it
                cc_out = dram.tile([n_cores, B, JD], F32,
                                   name=f"ccout_{tag}_{rep}",
                                   addr_space="Shared")
                nc.gpsimd.collective_compute(
                    "AllGather",
                    OP.bypass,
                    replica_groups=[list(range(n_cores))],
                    ins=[cc_in[:].opt()],
                    outs=[cc_out[:].opt()],
                )
                sg8 = small.tile([128, n_cores, JD], F32, tag="sg8")
                nc.sync.dma_start(
                    out=sg8[:],
                    in_=cc_out[:].rearrange("r b dj -> b r dj"))
                n = n_cores
                while n > 2:
                    h = n // 2
                    nc.vector.tensor_tensor(
                        sg8[:, 0:h, :], sg8[:, 0:h, :], sg8[:, h:n, :],
                        op=OP.add)
                    n = h
                nc.vector.tensor_tensor(
                    s_glob[:], sg8[:, 0, :], sg8[:, 1, :], op=OP.add)
                return s_glob
            cc_out = dram.tile([B, JD], F32, name=f"ccout_{tag}_{rep}",
                               addr_space="Shared")
            if use_cc:
                nc.gpsimd.collective_compute(
                    "AllReduce",
                    OP.add,
                    replica_groups=[list(range(n_cores))],
                    ins=[cc_in[:].opt()],
                    outs=[cc_out[:].opt()],
                )
            else:
                nc.sync.dma_start(out=cc_out[:], in_=cc_in[:])
            nc.sync.dma_start(out=s_glob[:], in_=cc_out[:])
            return s_glob

        s0g = all_reduce(s0p, "s0")  # overlaps the IH phase below

        # ---- materialize inputs_hat: IH[b, i, (d j)] bf16 --------------
        IH = big.tile([128, IL, JD], BF16)

        # drains must avoid gpsimd: its queue holds the collectives, so
        # Pool-side drains would stall behind a 32us AllReduce (and the
        # psIH pool would fill, stalling PE).
        def drain(k, dst, src):
            if k % 2 == 0:
                nc.scalar.copy(dst, src)
            else:
                nc.vector.tensor_copy(dst, src)

        kk = 0
        for g in range(G):
            for a in range(4):
                for h in range(2):
                    i0 = 16 * g + 4 * a + 2 * h
                    ps = psIH.tile([128, 2 * JD], F32, tag="ih")
                    nc.tensor.matmul(
                        ps[:], XT[32 * a:32 * a + 32, g, :],
                        WBD[32 * a:32 * a + 32, g,
                            2 * JD * h:2 * JD * (h + 1)],
                        start=True, stop=True, tile_position=(32 * a, 0))
                    drain(kk, IH[:, i0:i0 + 2, :], ps[:])
                    kk += 1

        # ---- routing helpers -------------------------------------------
        XB = big.tile([128, IL, JD], BF16)  # scratch for muls + trees
        L = big.tile([128, IL, J], BF16 if LBF else F32)  # routing logits
        spl = min(SPL, IL)
        iparts = [(nc.vector, 0, spl), (nc.gpsimd, spl, IL)]
        iparts = [(e, lo, hi) for e, lo, hi in iparts if hi > lo]

        def tree(eng, lo, n):
            """Halving-tree sum of XB rows [lo, lo+n) into row lo."""
            while n > 1:
                h = n // 2
                eng.tensor_tensor(
                    XB[:, lo:lo + h, :], XB[:, lo:lo + h, :],
                    XB[:, lo + h:lo + 2 * h, :], op=OP.add)
                if n % 2:
                    eng.tensor_tensor(
                        XB[:, lo:lo + 1, :], XB[:, lo:lo + 1, :],
                        XB[:, lo + n - 1:lo + n, :], op=OP.add)
                n = h

        def squash(s_glob):
            """squash along d of s_glob[128,(d j)] -> bf16 [128,(d j)].

            One ACT Sqrt (its table load has no data deps, so it hides
            under the preceding compute window); rest on DVE."""
            sq = small.tile([128, JD], F32, tag="sq")
            nc.vector.tensor_mul(sq[:], s_glob[:], s_glob[:])
            s2 = small.tile([128, J], F32, tag="s2")
            nc.vector.reduce_sum(
                s2[:], sq.rearrange("b (d j) -> b j d", d=D, j=J), axis=AX.X)
            # scale = s2 / ((1+s2) * sqrt(s2+eps));
            # 1/sqrt via exp(-0.5*ln) so ACT stays on the ln+exp table set
            if os.environ.get("K_SQRT", "0") == "1":
                t = small.tile([128, J], F32, tag="t")
                nc.scalar.activation(t[:], s2[:], AF.Sqrt, bias=eps_t[:])
                u = small.tile([128, J], F32, tag="u")
                nc.vector.tensor_scalar_add(u[:], s2[:], 1.0)
                w = small.tile([128, J], F32, tag="w")
                nc.vector.tensor_mul(w[:], u[:], t[:])
                rw = small.tile([128, J], F32, tag="rw")
                nc.vector.reciprocal(rw[:], w[:])
                sc = small.tile([128, J], F32, tag="sc")
                nc.vector.tensor_mul(sc[:], s2[:], rw[:])
            else:
                lt = small.tile([128, J], F32, tag="lt")
                nc.scalar.activation(lt[:], s2[:], AF.Ln, bias=eps_t[:])
                rt = small.tile([128, J], F32, tag="rt")
                nc.scalar.activation(rt[:], lt[:], AF.Exp, scale=-0.5)
                u = small.tile([128, J], F32, tag="u")
                nc.vector.tensor_scalar_add(u[:], s2[:], 1.0)
                rw = small.tile([128, J], F32, tag="rw")
                nc.vector.reciprocal(rw[:], u[:])
                sc = small.tile([128, J], F32, tag="sc")
                nc.vector.tensor_mul(sc[:], s2[:], rw[:])
                nc.vector.tensor_mul(sc[:], sc[:], rt[:])
            # o only feeds the bf16 agreement input -> emit bf16 directly
            o_b = small.tile([128, JD], BF16, tag="ob")
            sc_b = sc[:].unsqueeze(1).broadcast_to([128, D, J])
            nc.vector.tensor_tensor(
                o_b.rearrange("b (d j) -> b d j", d=D, j=J),
                s_glob.rearrange("b (d j) -> b d j", d=D, j=J),
                sc_b, op=OP.mult)
            return o_b

        def agr_range(eng, lo, hi, o_b, first):
            n_i = hi - lo
            xb = XB[:, lo:hi, :]
            eng.tensor_tensor(
                xb, IH[:, lo:hi, :],
                o_b[:].unsqueeze(1).broadcast_to([128, n_i, JD]),
                op=OP.mult)
            w = JD
            while w > 2 * J:
                h = w // 2
                eng.tensor_tensor(
                    xb[:, :, 0:h], xb[:, :, 0:h], xb[:, :, h:w],
                    op=OP.add)
                w = h
            if first:
                eng.tensor_tensor(
                    L[:, lo:hi, :], xb[:, :, 0:J], xb[:, :, J:2 * J],
                    op=OP.add)
            else:
                a1 = big.tile([128, IL, J], BF16 if LBF else F32,
                              tag="a1")
                eng.tensor_tensor(
                    a1[:, lo:hi, :], xb[:, :, 0:J], xb[:, :, J:2 * J],
                    op=OP.add)
                eng.tensor_tensor(
                    L[:, lo:hi, :], L[:, lo:hi, :], a1[:, lo:hi, :],
                    op=OP.add)

        def agreement(o_b, first):
            """b-logits += sum_d IH * o.

            Processed in i-halves matching softmax's split, so L[0:IL/2]
            completes early and ACT's exp of half 0 overlaps the DVE
            mul/tree of half 1."""
            for eng, lo0, hi0 in iparts:
                mid = (lo0 + hi0) // 2
                sub = ([(lo0, mid), (mid, hi0)]
                       if hi0 - lo0 >= 32 else [(lo0, hi0)])
                for lo, hi in sub:
                    agr_range(eng, lo, hi, o_b, first)

        def softmax():
            """c = softmax_j(L) -> bf16 [128, IL, J].

            Split in i-halves so DVE's Z/R/Cb chain on half 0 overlaps
            ACT's exp of half 1."""
            E = big.tile([128, IL, J], BF16 if EBF else F32, tag="E")
            Z = small.tile([128, IL], F32, tag="Z")
            R = small.tile([128, IL], BF16 if EBF else F32, tag="R")
            Cb = big.tile([128, IL, J], BF16, tag="Cb")
            halves = [(0, IL // 2), (IL // 2, IL)]
            for lo, hi in halves:
                nc.scalar.activation(E[:, lo:hi, :], L[:, lo:hi, :], AF.Exp)
            for lo, hi in halves:
                nc.vector.reduce_sum(Z[:, lo:hi], E[:, lo:hi, :], axis=AX.X)
                nc.vector.reciprocal(R[:, lo:hi], Z[:, lo:hi])
                nc.vector.tensor_tensor(
                    Cb[:, lo:hi, :], E[:, lo:hi, :],
                    R[:, lo:hi].unsqueeze(2).broadcast_to(
                        [128, hi - lo, J]),
                    op=OP.mult)
            return Cb

        def weighted_sum(Cb):
            """XB = IH * c (bcast over d); tree-reduce i -> s_part f32.

            Muls run per i-half so the first starts as soon as softmax's
            half 0 lands; tree roots fold directly into s_part."""
            XBv = XB.rearrange("b i (d j) -> b i d j", d=D, j=J)
            IHv = IH.rearrange("b i (d j) -> b i d j", d=D, j=J)
            Cbv = Cb[:].unsqueeze(2).broadcast_to([128, IL, D, J])
            roots = []
            for eng, lo0, hi0 in iparts:
                mid = (lo0 + hi0) // 2
                sub = ([(lo0, mid), (mid, hi0)]
                       if hi0 - lo0 >= 32 else [(lo0, hi0)])
                for lo, hi in sub:
                    eng.tensor_tensor(
                        XBv[:, lo:hi], IHv[:, lo:hi], Cbv[:, lo:hi],
                        op=OP.mult)
                for lo, hi in sub:
                    tree(eng, lo, hi - lo)
                    roots.append(lo)
            s_part = small.tile([128, JD], F32, tag="spart")
            if len(roots) >= 2:
                nc.vector.tensor_tensor(
                    s_part[:], XB[:, roots[0], :], XB[:, roots[1], :],
                    op=OP.add)
                for r in roots[2:]:
                    nc.vector.tensor_tensor(
                        s_part[:], s_part[:], XB[:, r, :], op=OP.add)
            else:
                nc.vector.tensor_copy(s_part[:], XB[:, roots[0], :])
            return s_part

        # ---- routing ----------------------------------------------------
        ob0 = squash(s0g)
        agreement(ob0, first=True)
        Cb = softmax()
        # iter 1
        s1p = weighted_sum(Cb)
        s1g = all_reduce(s1p, "s1")
        ob1 = squash(s1g)
        agreement(ob1, first=False)
        Cb = softmax()
        # iter 2: local partial only; host sums across cores + squashes
        s2p = weighted_sum(Cb)
        nc.sync.dma_start(out=out_d[:], in_=s2p[:])


def make_in_maps(inputs, W):
    """Host-side shard + pre-transpose + bf16 cast of the full inputs."""
    import ml_dtypes

    x = np.ascontiguousarray(np.asarray(inputs), dtype=np.float32)
    W0 = np.ascontiguousarray(np.asarray(W), dtype=np.float32)
    if W0.ndim == 5:
        W0 = W0[0]
    IL = I_FULL // NCORES
    G = IL // 16
    bf = ml_dtypes.bfloat16
    in_maps = []
    for c in range(NCORES):
        xc = x[:, c * IL:(c + 1) * IL, :]              # [B, IL, P]
        Wc = W0[c * IL:(c + 1) * IL]                   # [IL, J, D, P]
        # xt[k=(i16 p), g, b]
        xt = xc.reshape(B, G, 16, P_DIM).transpose(2, 3, 1, 0).reshape(
            128, G, B)
        # w2[k, g, (d j)]
        w2 = Wc.reshape(G, 16, J, D, P_DIM).transpose(1, 4, 0, 3, 2).reshape(
            128, G, JD)
        # wbd[k, g, 4*JD]: block-diagonal expansion, 4 i's per K=32 slice
        wbd = np.zeros((128, G, 4 * JD), np.float32)
        for t in range(4):
            for a in range(4):
                r0 = 32 * a + 8 * t
                wbd[r0:r0 + 8, :, JD * t:JD * (t + 1)] = w2[r0:r0 + 8]
        in_maps.append({
            "xt": np.ascontiguousarray(xt.astype(bf)),
            "w2": np.ascontiguousarray(w2.astype(bf)),
            "wbd": np.ascontiguousarray(wbd.astype(bf)),
        })
    return in_maps


def _host_finish(parts):
    """Sum per-core partial s2 [B, (d j)] and apply squash -> [B, J, D]."""
    s = np.zeros((B, JD), np.float64)
    for p in parts:
        s += np.asarray(p, dtype=np.float64)
    s = s.reshape(B, D, J).transpose(0, 2, 1)  # [B, J, D]
    s2 = np.sum(s * s, axis=-1, keepdims=True)
    out = s2 / (1.0 + s2) / np.sqrt(s2 + EPS) * s
    return out.astype(np.float32)


@functools.lru_cache(maxsize=None)
def _get_nc():
    return build(NCORES, I_FULL // NCORES)


def kernel(inputs, W):
    """Full-input entry point: inputs [128,1152,8] f32, W [1,1152,10,16,8]."""
    from concourse.bass_utils import run_bass_kernel_spmd

    nc = _get_nc()
    in_maps = make_in_maps(inputs, W)
    res = run_bass_kernel_spmd(nc, in_maps, core_ids=list(range(NCORES)))
    return _host_finish([r["out"] for r in res.results])


if __name__ == "__main__":
    nc = build(1, 16)
    print("built OK")


# revision 38
# speedup vs baseline: 1.0122x; 1.0122x over previous
"""Trainium2 Bass kernel for nn_CapsuleLayer (dynamic routing).

Reference computation (B=128, I=1152, P=8, J=10, D=16):
    inputs_hat[b,i,j,d] = sum_p W[i,j,d,p] * inputs[b,i,p]
    b_logits = 0
    3x routing:
        c = softmax_j(b_logits)
        s[b,j,d] = sum_i c[b,i,j] * inputs_hat[b,i,j,d]
        outputs = squash(s)
        b_logits += sum_d inputs_hat[b,i,j,d] * outputs[b,j,d]   (iters 0,1)

Distribution: i-sharded across 8 cores (IL=144 i's per core), full batch
B=128 in the 128 SBUF partitions on every core.  Cross-core traffic is an
80KB AllReduce of the s-partials for routing iterations 0 and 1; the final
iteration's partial sum is returned per-core and reduced + squashed on the
host during the gather/unshard step (saves the third collective).

Host-side prep (part of sharding): x and W are pre-transposed and cast to
bf16 on the host into the exact SBUF layouts the PE needs —
  xt  [k=(i16 p)=128, g, b]      stationary operand k-tiles
  w2  [k, g, (d j)=160]          dense W for the iteration-0 s matmul
  wbd [k, g, 4*JD=640]           block-diagonal W expansion (4 i's per
                                 K=32 slice) for the inputs_hat matmuls
so the device does no staging at all: 3 DMA loads, then straight to
matmuls.  bf16 operands run the PE at 1 cycle/row (fp32 is 4).

Engine budget: PE does s0 + inputs_hat; DVE runs ALL routing elementwise
work (bf16 at 2x; real-HW gpsimd elementwise is far slower than the cost
model claims, so no Pool offload); ACT handles exp/sqrt, half the PSUM
drains, and scaling; the collectives are issued from gpsimd (only engine
that can) — its queue blocking during the collective aligns with the data
dependency anyway; SP triggers the bounce DMAs.
"""

import os
import sys
import functools

import numpy as np

if "/opt/trn_rl_repo" not in sys.path:
    sys.path.insert(0, "/opt/trn_rl_repo")

B = 128
I_FULL = 1152
P_DIM = 8
J = 10
D = 16
JD = D * J  # 160, flattened (d, j): col = d*J + j
NCORES = 8
ROUTINGS = 3
EPS = 1e-7

# elementwise split: DVE takes i-rows [0, SPL), Pool takes [SPL, IL).
# SPL=IL disables the Pool offload: on real HW gpsimd tensor_tensor is
# far slower than the cost model's 0.83 ns/elem (measured: offloading a
# third of the muls to Pool ADDED ~68us), so everything stays on DVE.
SPL = int(os.environ.get("K_SPL", "144"))
LBF = os.environ.get("K_LBF", "0") == "1"   # bf16 routing logits
EBF = os.environ.get("K_EBF", "0") == "1"   # bf16 softmax numerator
AGX = os.environ.get("K_AG", "0") == "1"    # AllGather + local sum


def build(n_cores, IL, repeat=1):
    """Trace + compile the SPMD Bass program (one program, all cores)."""
    import concourse.bacc as bacc
    import concourse.bass as bass
    import concourse.mybir as mybir
    import concourse.tile as tile
    from concourse.masks import make_identity

    F32 = mybir.dt.float32
    BF16 = mybir.dt.bfloat16

    assert IL % 16 == 0
    G = IL // 16

    nc = bacc.Bacc(
        "TRN2", target_bir_lowering=False, debug=False, num_devices=n_cores
    )
    xt_d = nc.dram_tensor("xt", [128, G, 128], BF16, kind="ExternalInput").ap()
    w2_d = nc.dram_tensor("w2", [128, G, JD], BF16, kind="ExternalInput").ap()
    wbd_d = nc.dram_tensor(
        "wbd", [128, G, 4 * JD], BF16, kind="ExternalInput").ap()
    out_d = nc.dram_tensor("out", [B, JD], F32, kind="ExternalOutput").ap()

    import contextlib as _ctxlib
    lp = (nc.allow_low_precision(reason="bf16 routing-state experiment")
          if (LBF or EBF) else _ctxlib.nullcontext())
    with tile.TileContext(nc, num_cores=n_cores) as tc, lp:
        for rep in range(repeat):
            _trace(tc, nc, xt_d, w2_d, wbd_d, out_d, n_cores, IL, G,
                   mybir, make_identity, rep)

    nc.compile()
    return nc


def _trace(tc, nc, xt_d, w2_d, wbd_d, out_d, n_cores, IL, G, mybir,
           make_identity, rep=0):
    import contextlib

    F32 = mybir.dt.float32
    BF16 = mybir.dt.bfloat16
    AF = mybir.ActivationFunctionType
    OP = mybir.AluOpType
    AX = mybir.AxisListType

    ctx = contextlib.ExitStack()
    with ctx:
        singles = ctx.enter_context(
            tc.tile_pool(name=f"singles{rep}", bufs=1))
        big = ctx.enter_context(tc.tile_pool(name=f"big{rep}", bufs=1))
        small = ctx.enter_context(tc.tile_pool(name=f"small{rep}", bufs=3))
        psT = ctx.enter_context(
            tc.tile_pool(name=f"psT{rep}", bufs=2, space="PSUM"))
        psS = ctx.enter_context(
            tc.tile_pool(name=f"psS{rep}", bufs=1, space="PSUM"))
        psIH = ctx.enter_context(
            tc.tile_pool(name=f"psIH{rep}", bufs=4, space="PSUM"))
        dram = ctx.enter_context(
            tc.tile_pool(name=f"dram{rep}", bufs=1, space="DRAM"))

        # ---- constants -------------------------------------------------
        ident = singles.tile([128, 128], F32)
        make_identity(nc, ident[:])
        dummy = singles.tile([128, 1], F32)
        nc.vector.memset(dummy[:], 0.0)
        eps_t = singles.tile([128, 1], F32)
        nc.vector.memset(eps_t[:], EPS)
        # Optional manual preload of the natural_log_exp table set. In the
        # cost model this reduces the program to ONE table load, but on
        # real HW the Ln/Exp+preload variant measured SLOWER than the
        # Sqrt-based squash (201.0 vs 186.2us back-to-back), so both
        # knobs default to the measured-best configuration.
        if os.environ.get("K_MANLOAD", "0") == "1":
            from concourse.hw_specs import get_activation_tables
            tabs = get_activation_tables(nc.m.arch)
            set_id = list(tabs).index("natural_log_exp_and_others")
            nc.scalar.add_instruction(mybir.InstLoadActFuncSet(
                name=f"I-{nc.next_id()}", ins=[], outs=[],
                act_func_set_id=set_id))
        nc.scalar.activation(dummy[:], dummy[:], AF.Exp)

        # ---- load pre-transposed operands (3 g-groups, 3 queues) -------
        XT = big.tile([128, G, 128], BF16)
        W2 = big.tile([128, G, JD], BF16)
        WBD = big.tile([128, G, 4 * JD], BF16)
        gparts = [(i, min(i + 3, G)) for i in range(0, G, 3)]
        for g0, g1 in gparts:
            nc.sync.dma_start(out=XT[:, g0:g1, :], in_=xt_d[:, g0:g1, :])
            nc.scalar.dma_start(out=W2[:, g0:g1, :], in_=w2_d[:, g0:g1, :])
            nc.gpsimd.dma_start(out=WBD[:, g0:g1, :], in_=wbd_d[:, g0:g1, :])

        # warm the PE p-state ramp (0.65 -> 2.4 GHz needs ~3us of
        # continuous execution) with dependency-free identity transposes
        # while the input DMAs land
        for _ in range(6):
            ps_w = psT.tile([128, 128], F32, tag="pst")
            nc.tensor.transpose(ps_w[:], ident[:], ident[:])

        # ---- iteration-0 s directly from PE (c == 1/J), AllReduce ------
        # s0T[(d j), b] = sum_{(i,p)} W2[k, dj] * XT[k, b]
        ps_a = psS.tile([128, 128], F32, tag="s0a")
        ps_b = psS.tile([32, 128], F32, tag="s0b")
        for g in range(G):
            nc.tensor.matmul(ps_a[:], W2[:, g, 0:128], XT[:, g, :],
                             start=(g == 0), stop=(g == G - 1))
        for g in range(G):
            nc.tensor.matmul(ps_b[:], W2[:, g, 128:JD], XT[:, g, :],
                             start=(g == 0), stop=(g == G - 1))
        s0T_a = small.tile([128, 128], F32, tag="s0Ta")
        s0T_b = small.tile([32, 128], F32, tag="s0Tb")
        nc.scalar.mul(s0T_a[:], ps_a[:], 1.0 / J)
        nc.scalar.mul(s0T_b[:], ps_b[:], 1.0 / J)
        s0p = small.tile([128, JD], F32, tag="spart")
        pst = psT.tile([128, 128], F32, tag="pst")
        nc.tensor.transpose(pst[:], s0T_a[:], ident[:])
        nc.vector.tensor_copy(s0p[:, 0:128], pst[:])
        pstb = psT.tile([128, 32], F32, tag="pst")
        nc.tensor.transpose(pstb[:], s0T_b[:], ident[0:32, 0:32])
        nc.vector.tensor_copy(s0p[:, 128:JD], pstb[:])

        def all_reduce(s_part, tag):
            use_cc = (n_cores > 1
                      and os.environ.get("K_NO_CC", "0") != "1")
            s_glob = small.tile([128, JD], F32, tag="sglob")
            cc_in = dram.tile([B, JD], F32, name=f"ccin_{tag}_{rep}")
            nc.sync.dma_start(out=cc_in[:], in_=s_part[:])
            if use_cc and AGX:
                # AllGather + local 8-slot tree sum: real-HW AllReduce is
                # two-phase; a gather plus ~0.7us of DVE adds may beat it
                cc_out = dram.tile([n_cores, B, JD], F32,
                                   name=f"ccout_{tag}_{rep}",
                                   addr_space="Shared")
                nc.gpsimd.collective_compute(
                    "AllGather",
                    OP.bypass,
                    replica_groups=[list(range(n_cores))],
                    ins=[cc_in[:].opt()],
                    outs=[cc_out[:].opt()],
                )
                sg8 = small.tile([128, n_cores, JD], F32, tag="sg8")
                nc.sync.dma_start(
                    out=sg8[:],
                    in_=cc_out[:].rearrange("r b dj -> b r dj"))
                n = n_cores
                while n > 2:
                    h = n // 2
                    nc.vector.tensor_tensor(
                        sg8[:, 0:h, :], sg8[:, 0:h, :], sg8[:, h:n, :],
                        op=OP.add)
                    n = h
                nc.vector.tensor_tensor(
                    s_glob[:], sg8[:, 0, :], sg8[:, 1, :], op=OP.add)
                return s_glob
            cc_out = dram.tile([B, JD], F32, name=f"ccout_{tag}_{rep}",
                               addr_space="Shared")
            if use_cc:
                nc.gpsimd.collective_compute(
                    "AllReduce",
                    OP.add,
                    replica_groups=[list(range(n_cores))],
                    ins=[cc_in[:].opt()],
                    outs=[cc_out[:].opt()],
                )
            else:
                nc.sync.dma_start(out=cc_out[:], in_=cc_in[:])
            nc.sync.dma_start(out=s_glob[:], in_=cc_out[:])
            return s_glob

        s0g = all_reduce(s0p, "s0")  # overlaps the IH phase below

        # ---- materialize inputs_hat: IH[b, i, (d j)] bf16 --------------
        IH = big.tile([128, IL, JD], BF16)

        # drains must avoid gpsimd: its queue holds the collectives, so
        # Pool-side drains would stall behind a 32us AllReduce (and the
        # psIH pool would fill, stalling PE).
        def drain(k, dst, src):
            if k % 2 == 0:
                nc.scalar.copy(dst, src)
            else:
                nc.vector.tensor_copy(dst, src)

        kk = 0
        for g in range(G):
            for a in range(4):
                for h in range(2):
                    i0 = 16 * g + 4 * a + 2 * h
                    ps = psIH.tile([128, 2 * JD], F32, tag="ih")
                    nc.tensor.matmul(
                        ps[:], XT[32 * a:32 * a + 32, g, :],
                        WBD[32 * a:32 * a + 32, g,
                            2 * JD * h:2 * JD * (h + 1)],
                        start=True, stop=True, tile_position=(32 * a, 0))
                    drain(kk, IH[:, i0:i0 + 2, :], ps[:])
                    kk += 1

        # ---- routing helpers -------------------------------------------
        XB = big.tile([128, IL, JD], BF16)  # scratch for muls + trees
        L = big.tile([128, IL, J], BF16 if LBF else F32)  # routing logits
        spl = min(SPL, IL)
        iparts = [(nc.vector, 0, spl), (nc.gpsimd, spl, IL)]
        iparts = [(e, lo, hi) for e, lo, hi in iparts if hi > lo]

        def tree(eng, lo, n):
            """Halving-tree sum of XB rows [lo, lo+n) into row lo."""
            while n > 1:
                h = n // 2
                eng.tensor_tensor(
                    XB[:, lo:lo + h, :], XB[:, lo:lo + h, :],
                    XB[:, lo + h:lo + 2 * h, :], op=OP.add)
                if n % 2:
                    eng.tensor_tensor(
                        XB[:, lo:lo + 1, :], XB[:, lo:lo + 1, :],
                        XB[:, lo + n - 1:lo + n, :], op=OP.add)
                n = h

        def squash(s_glob):
            """squash along d of s_glob[128,(d j)] -> bf16 [128,(d j)].

            One ACT Sqrt (its table load has no data deps, so it hides
            under the preceding compute window); rest on DVE."""
            sq = small.tile([128, JD], F32, tag="sq")
            nc.vector.tensor_mul(sq[:], s_glob[:], s_glob[:])
            s2 = small.tile([128, J], F32, tag="s2")
            nc.vector.reduce_sum(
                s2[:], sq.rearrange("b (d j) -> b j d", d=D, j=J), axis=AX.X)
            # scale = s2 / ((1+s2) * sqrt(s2+eps));
            # one ACT Sqrt measured faster on HW than the Ln/Exp pair
            if os.environ.get("K_SQRT", "1") == "1":
                t = small.tile([128, J], F32, tag="t")
                nc.scalar.activation(t[:], s2[:], AF.Sqrt, bias=eps_t[:])
                u = small.tile([128, J], F32, tag="u")
                nc.vector.tensor_scalar_add(u[:], s2[:], 1.0)
                w = small.tile([128, J], F32, tag="w")
                nc.vector.tensor_mul(w[:], u[:], t[:])
                rw = small.tile([128, J], F32, tag="rw")
                nc.vector.reciprocal(rw[:], w[:])
                sc = small.tile([128, J], F32, tag="sc")
                nc.vector.tensor_mul(sc[:], s2[:], rw[:])
            else:
                lt = small.tile([128, J], F32, tag="lt")
                nc.scalar.activation(lt[:], s2[:], AF.Ln, bias=eps_t[:])
                rt = small.tile([128, J], F32, tag="rt")
                nc.scalar.activation(rt[:], lt[:], AF.Exp, scale=-0.5)
                u = small.tile([128, J], F32, tag="u")
                nc.vector.tensor_scalar_add(u[:], s2[:], 1.0)
                rw = small.tile([128, J], F32, tag="rw")
                nc.vector.reciprocal(rw[:], u[:])
                sc = small.tile([128, J], F32, tag="sc")
                nc.vector.tensor_mul(sc[:], s2[:], rw[:])
                nc.vector.tensor_mul(sc[:], sc[:], rt[:])
            # o only feeds the bf16 agreement input -> emit bf16 directly
            o_b = small.tile([128, JD], BF16, tag="ob")
            sc_b = sc[:].unsqueeze(1).broadcast_to([128, D, J])
            nc.vector.tensor_tensor(
                o_b.rearrange("b (d j) -> b d j", d=D, j=J),
                s_glob.rearrange("b (d j) -> b d j", d=D, j=J),
                sc_b, op=OP.mult)
            return o_b

        def agr_range(eng, lo, hi, o_b, first):
            n_i = hi - lo
            xb = XB[:, lo:hi, :]
            eng.tensor_tensor(
                xb, IH[:, lo:hi, :],
                o_b[:].unsqueeze(1).broadcast_to([128, n_i, JD]),
                op=OP.mult)
            w = JD
            while w > 2 * J:
                h = w // 2
                eng.tensor_tensor(
                    xb[:, :, 0:h], xb[:, :, 0:h], xb[:, :, h:w],
                    op=OP.add)
                w = h
            if first:
                eng.tensor_tensor(
                    L[:, lo:hi, :], xb[:, :, 0:J], xb[:, :, J:2 * J],
                    op=OP.add)
            else:
                a1 = big.tile([128, IL, J], BF16 if LBF else F32,
                              tag="a1")
                eng.tensor_tensor(
                    a1[:, lo:hi, :], xb[:, :, 0:J], xb[:, :, J:2 * J],
                    op=OP.add)
                eng.tensor_tensor(
                    L[:, lo:hi, :], L[:, lo:hi, :], a1[:, lo:hi, :],
                    op=OP.add)

        def agreement(o_b, first):
            """b-logits += sum_d IH * o.

            Processed in i-halves matching softmax's split, so L[0:IL/2]
            completes early and ACT's exp of half 0 overlaps the DVE
            mul/tree of half 1."""
            for eng, lo0, hi0 in iparts:
                mid = (lo0 + hi0) // 2
                sub = ([(lo0, mid), (mid, hi0)]
                       if hi0 - lo0 >= 32 else [(lo0, hi0)])
                for lo, hi in sub:
                    agr_range(eng, lo, hi, o_b, first)

        def softmax():
            """c = softmax_j(L) -> bf16 [128, IL, J].

            Split in i-halves so DVE's Z/R/Cb chain on half 0 overlaps
            ACT's exp of half 1."""
            E = big.tile([128, IL, J], BF16 if EBF else F32, tag="E")
            Z = small.tile([128, IL], F32, tag="Z")
            R = small.tile([128, IL], BF16 if EBF else F32, tag="R")
            Cb = big.tile([128, IL, J], BF16, tag="Cb")
            halves = [(0, IL // 2), (IL // 2, IL)]
            for lo, hi in halves:
                nc.scalar.activation(E[:, lo:hi, :], L[:, lo:hi, :], AF.Exp)
            for lo, hi in halves:
                nc.vector.reduce_sum(Z[:, lo:hi], E[:, lo:hi, :], axis=AX.X)
                nc.vector.reciprocal(R[:, lo:hi], Z[:, lo:hi])
                nc.vector.tensor_tensor(
                    Cb[:, lo:hi, :], E[:, lo:hi, :],
                    R[:, lo:hi].unsqueeze(2).broadcast_to(
                        [128, hi - lo, J]),
                    op=OP.mult)
            return Cb

        def weighted_sum(Cb):
            """XB = IH * c (bcast over d); tree-reduce i -> s_part f32.

            Muls run per i-half so the first starts as soon as softmax's
            half 0 lands; tree roots fold directly into s_part."""
            XBv = XB.rearrange("b i (d j) -> b i d j", d=D, j=J)
            IHv = IH.rearrange("b i (d j) -> b i d j", d=D, j=J)
            Cbv = Cb[:].unsqueeze(2).broadcast_to([128, IL, D, J])
            roots = []
            for eng, lo0, hi0 in iparts:
                mid = (lo0 + hi0) // 2
                sub = ([(lo0, mid), (mid, hi0)]
                       if hi0 - lo0 >= 32 else [(lo0, hi0)])
                for lo, hi in sub:
                    eng.tensor_tensor(
                        XBv[:, lo:hi], IHv[:, lo:hi], Cbv[:, lo:hi],
                        op=OP.mult)
                for lo, hi in sub:
                    tree(eng, lo, hi - lo)
                    roots.append(lo)
            s_part = small.tile([128, JD], F32, tag="spart")
            if len(roots) >= 2:
                nc.vector.tensor_tensor(
                    s_part[:], XB[:, roots[0], :], XB[:, roots[1], :],
                    op=OP.add)
                for r in roots[2:]:
                    nc.vector.tensor_tensor(
                        s_part[:], s_part[:], XB[:, r, :], op=OP.add)
            else:
                nc.vector.tensor_copy(s_part[:], XB[:, roots[0], :])
            return s_part

        # ---- routing ----------------------------------------------------
        ob0 = squash(s0g)
        agreement(ob0, first=True)
        Cb = softmax()
        # iter 1
        s1p = weighted_sum(Cb)
        s1g = all_reduce(s1p, "s1")
        ob1 = squash(s1g)
        agreement(ob1, first=False)
        Cb = softmax()
        # iter 2: local partial only; host sums across cores + squashes
        s2p = weighted_sum(Cb)
        nc.sync.dma_start(out=out_d[:], in_=s2p[:])


def make_in_maps(inputs, W):
    """Host-side shard + pre-transpose + bf16 cast of the full inputs."""
    import ml_dtypes

    x = np.ascontiguousarray(np.asarray(inputs), dtype=np.float32)
    W0 = np.ascontiguousarray(np.asarray(W), dtype=np.float32)
    if W0.ndim == 5:
        W0 = W0[0]
    IL = I_FULL // NCORES
    G = IL // 16
    bf = ml_dtypes.bfloat16
    in_maps = []
    for c in range(NCORES):
        xc = x[:, c * IL:(c + 1) * IL, :]              # [B, IL, P]
        Wc = W0[c * IL:(c + 1) * IL]                   # [IL, J, D, P]
        # xt[k=(i16 p), g, b]
        xt = xc.reshape(B, G, 16, P_DIM).transpose(2, 3, 1, 0).reshape(
            128, G, B)
        # w2[k, g, (d j)]
        w2 = Wc.reshape(G, 16, J, D, P_DIM).transpose(1, 4, 0, 3, 2).reshape(
            128, G, JD)
        # wbd[k, g, 4*JD]: block-diagonal expansion, 4 i's per K=32 slice
        wbd = np.zeros((128, G, 4 * JD), np.float32)
        for t in range(4):
            for a in range(4):
                r0 = 32 * a + 8 * t
                wbd[r0:r0 + 8, :, JD * t:JD * (t + 1)] = w2[r0:r0 + 8]
        in_maps.append({
            "xt": np.ascontiguousarray(xt.astype(bf)),
            "w2": np.ascontiguousarray(w2.astype(bf)),
            "wbd": np.ascontiguousarray(wbd.astype(bf)),
        })
    return in_maps


def _host_finish(parts):
    """Sum per-core partial s2 [B, (d j)] and apply squash -> [B, J, D]."""
    s = np.zeros((B, JD), np.float64)
    for p in parts:
        s += np.asarray(p, dtype=np.float64)
    s = s.reshape(B, D, J).transpose(0, 2, 1)  # [B, J, D]
    s2 = np.sum(s * s, axis=-1, keepdims=True)
    out = s2 / (1.0 + s2) / np.sqrt(s2 + EPS) * s
    return out.astype(np.float32)


@functools.lru_cache(maxsize=None)
def _get_nc():
    return build(NCORES, I_FULL // NCORES)


def kernel(inputs, W):
    """Full-input entry point: inputs [128,1152,8] f32, W [1,1152,10,16,8]."""
    from concourse.bass_utils import run_bass_kernel_spmd

    nc = _get_nc()
    in_maps = make_in_maps(inputs, W)
    res = run_bass_kernel_spmd(nc, in_maps, core_ids=list(range(NCORES)))
    return _host_finish([r["out"] for r in res.results])


if __name__ == "__main__":
    nc = build(1, 16)
    print("built OK")
